# revision 6
# baseline (speedup 1.0000x reference)
"""GQA causal attention (B=1, S=2048, D=4096, H=32, KV=8) on 8 trn2 cores.

Strategy: tensor-parallel over heads for QKV+attention, tensor-parallel
over output columns for the out-projection. Core i owns q-heads 4i..4i+3
and kv-head i. Host pre-transposes weights/x so every matmul contracts
along the partition dim, and pre-permutes wq/wk rows (even|odd
interleave -> [evens;odds]) so RoPE becomes partition-aligned
elementwise math.

Phase 1 (QKV+RoPE) streams x do-tile-major with six concurrent PSUM
accumulation groups so compute starts as soon as the first 128-row
chunk of x/w lands. Attention runs head-locally in a scores^T [t, s]
layout with a two-deep score-matmul pipeline ahead of the exp/mask
chain (exp on Scalar, causal mask multiply on GpSimd, softmax
normalization sums from a ones-vector matmul). Each block's normalized
head outputs are AllGathered (bf16); every core then computes its
512-column shard of the output projection for those tokens using all
32 heads, overlapped so the AllGather of block k hides under the
attention of block k+1 and the out-projection of block k-1. Host
concatenates column shards.

Matmul operands are bf16; accumulation, softmax and RoPE math are fp32.
"""

import sys

import numpy as np

sys.path.insert(0, "/opt/trn_rl_repo")

import ml_dtypes  # noqa: E402

import concourse.bass as bass  # noqa: E402
from concourse import bacc  # noqa: E402
import concourse.mybir as mybir  # noqa: E402
import concourse.tile as tile  # noqa: E402
from concourse.bass_utils import run_bass_kernel_spmd  # noqa: E402

F32 = mybir.dt.float32
BF16 = mybir.dt.bfloat16
NPBF = ml_dtypes.bfloat16

B, S, D = 1, 2048, 4096
H, KV, HD = 32, 8, 128
NCORES = 8
HPC = H // NCORES  # q heads per core = 4
EQ = HPC * HD  # 512 local q features
NE = HPC + 2  # e-tiles per core: 4 q + 1 k + 1 v
SB = 512  # attention s block
NSB = S // SB  # 4
XSB = 512  # phase-1 s sub-block (matmul moving dim)
DO = D // 128  # 32 contraction tiles for qkv projection
EO = (H * HD) // 128  # 32 contraction tiles for out-proj
DSH = D // NCORES  # 512 out-proj columns per core
TT = S // 128  # 16 t-tiles
DC = 4  # phase-1 do-tiles per DMA chunk
RG = [list(range(NCORES))]


def build():
    nc = bacc.Bacc("TRN2", target_bir_lowering=False)
    xt = nc.dram_tensor("xt", [D, S], BF16, kind="ExternalInput")
    wqkvt = nc.dram_tensor("wqkvt", [D, NE * 128], BF16, kind="ExternalInput")
    w3t = nc.dram_tensor("w3t", [H * HD, DSH], BF16, kind="ExternalInput")
    cc = nc.dram_tensor("cc", [128, S], F32, kind="ExternalInput")
    ss = nc.dram_tensor("ss", [128, S], F32, kind="ExternalInput")
    masks = nc.dram_tensor("masks", [HPC, 128, SB], BF16,
                           kind="ExternalInput")
    ones = nc.dram_tensor("ones", [128, 1], BF16, kind="ExternalInput")
    ident = nc.dram_tensor("ident", [128, 128], BF16, kind="ExternalInput")
    out = nc.dram_tensor("out", [S, DSH], BF16, kind="ExternalOutput")

    xt_t = xt[:].rearrange("(do p) s -> p do s", p=128)
    w_t = wqkvt[:].rearrange("(do p) e -> p do e", p=128)
    w3_t = w3t[:].rearrange("(eo p) d -> p eo d", p=128)

    with tile.TileContext(nc) as tc:
        with tc.tile_pool(name="dram", bufs=1, space="DRAM") as dram, \
                tc.tile_pool(name="pqkv", bufs=1) as pqkv:
            ag_in = [dram.tile([EQ, SB], BF16, name=f"agi{bi}")
                     for bi in range(NSB)]
            ag_out = [dram.tile([H * HD, SB], BF16, name=f"ago{bi}")
                      for bi in range(NSB)]
            qe = [pqkv.tile([128, S], BF16, name=f"qe{et}", tag=f"qe{et}")
                  for et in range(NE)]

            # ---------------- Phase 1: fused QKV projection + RoPE ----------
            with tc.tile_pool(name="p1w", bufs=1) as p1w, \
                    tc.tile_pool(name="p1x", bufs=2) as p1x, \
                    tc.tile_pool(name="p1t", bufs=2) as p1t, \
                    tc.tile_pool(name="p1ps", bufs=1, space="PSUM") as p1ps:
                w = p1w.tile([128, DO, NE * 128], BF16)
                xtiles = []
                # stream first two x blocks and weights in do-chunks so the
                # PE can start on do 0..3 while the rest is in flight; x on
                # the sync queue, w on the scalar queue (parallel arrival)
                for sb in range(2):
                    ssl = slice(sb * XSB, (sb + 1) * XSB)
                    xtile = p1x.tile([128, DO, XSB], BF16, tag="x")
                    for dcb in range(DO // DC):
                        dsl = slice(dcb * DC, (dcb + 1) * DC)
                        nc.sync.dma_start(xtile[:, dsl, :], xt_t[:, dsl, ssl])
                        if sb == 0:
                            nc.scalar.dma_start(w[:, dsl, :], w_t[:, dsl, :])
                    xtiles.append(xtile)
                for sb in range(NSB):
                    ssl = slice(sb * XSB, (sb + 1) * XSB)
                    if sb < 2:
                        xtile = xtiles[sb]
                    else:
                        xtile = p1x.tile([128, DO, XSB], BF16, tag="x")
                        nc.sync.dma_start(xtile, xt_t[:, :, ssl])
                    cct = p1x.tile([128, XSB], F32, tag="cc")
                    sst = p1x.tile([128, XSB], F32, tag="ss")
                    nc.sync.dma_start(cct, cc[:][:, ssl])
                    nc.sync.dma_start(sst, ss[:][:, ssl])
                    atile = p1t.tile([128, HPC + 1, XSB], F32, tag="at",
                                     bufs=1)
                    btile = p1t.tile([128, HPC + 1, XSB], F32, tag="bt",
                                     bufs=1)
                    bsw = p1t.tile([128, (HPC + 1) * XSB], F32, tag="bsw",
                                   bufs=1)
                    ps = [p1ps.tile([128, XSB], F32, tag=f"ps{et}",
                                    name=f"ps{et}")
                          for et in range(NE)]
                    # do-major: six concurrent PSUM groups; MMs for do-chunk
                    # d need only chunk d of x and w
                    for do in range(DO):
                        for et in range(NE):
                            nc.tensor.matmul(
                                ps[et],
                                w[:, do, et * 128:(et + 1) * 128],
                                xtile[:, do, :],
                                start=(do == 0),
                                stop=(do == DO - 1),
                            )
                    for et in range(NE):
                        if et < HPC + 1:
                            # t*cos and t*sin halves; swap+add finishes RoPE
                            nc.vector.tensor_mul(atile[:, et, :], ps[et], cct)
                            nc.vector.tensor_mul(btile[:, et, :], ps[et], sst)
                        else:
                            nc.any.tensor_copy(qe[et][:, ssl], ps[et])
                    bt2 = btile.rearrange("p e s -> p (e s)")
                    nc.sync.dma_start(bsw[:64, :], bt2[64:, :])
                    nc.sync.dma_start(bsw[64:, :], bt2[:64, :])
                    bs3 = bsw.rearrange("p (e s) -> p e s", s=XSB)
                    for et in range(HPC + 1):
                        nc.vector.tensor_add(
                            qe[et][:, ssl], atile[:, et, :], bs3[:, et, :])

            # ------- Phase 2+3: attention blocks + pipelined out-proj -------
            with tc.tile_pool(name="p2c", bufs=1) as p2c, \
                    tc.tile_pool(name="p2e", bufs=4) as p2e, \
                    tc.tile_pool(name="p2t", bufs=2) as p2t, \
                    tc.tile_pool(name="p2a", bufs=4) as p2a, \
                    tc.tile_pool(name="p3a", bufs=2) as p3a, \
                    tc.tile_pool(name="p3o", bufs=2) as p3o, \
                    tc.tile_pool(name="psS", bufs=2, space="PSUM") as psS, \
                    tc.tile_pool(name="pav", bufs=2, space="PSUM") as pavp, \
                    tc.tile_pool(name="psN", bufs=2, space="PSUM") as psN, \
                    tc.tile_pool(name="pop", bufs=2, space="PSUM") as popp:
                # tiny setup tensors first (sync queue) so the vn transposes
                # aren't stuck behind the big w3 stream (scalar queue)
                mt = p2c.tile([128, HPC, SB], BF16)
                nc.sync.dma_start(mt, masks[:].rearrange("m p s -> p m s"))
                on = p2c.tile([128, 1], BF16)
                nc.sync.dma_start(on, ones[:])
                idt = p2c.tile([128, 128], BF16)
                nc.sync.dma_start(idt, ident[:])
                w3sb = p2c.tile([128, EO, DSH], BF16)
                nc.scalar.dma_start(w3sb, w3_t)

                # v from [hd, s] e-tile layout to natural [t, hd] tiles
                vn = p2c.tile([128, TT, HD], BF16)
                for tt in range(TT):
                    pst = psS.tile([128, SB], BF16, tag="sc")
                    nc.tensor.transpose(
                        pst[:, :128],
                        qe[HPC + 1][:, tt * 128:(tt + 1) * 128], idt)
                    nc.any.tensor_copy(vn[:, tt, :], pst[:, :128])

                def outproj(bi):
                    bb = 3 - bi  # block id of buffer bi
                    aog = p3a.tile([128, EO, SB], BF16, tag="aog")
                    nc.sync.dma_start(
                        aog, ag_out[bi][:].rearrange("(eo p) s -> p eo s",
                                                     p=128))
                    for stl in range(4):
                        st = bb * 4 + stl
                        po = popp.tile([128, DSH], F32, tag="pop")
                        for eo in range(EO):
                            nc.tensor.matmul(
                                po,
                                aog[:, eo, stl * 128:(stl + 1) * 128],
                                w3sb[:, eo, :],
                                start=(eo == 0), stop=(eo == EO - 1))
                        osb = p3o.tile([128, DSH], BF16, tag="osb")
                        nc.vector.tensor_copy(osb, po)
                        nc.sync.dma_start(
                            out[:][st * 128:(st + 1) * 128, :], osb)

                for bi, b in enumerate((3, 2, 1, 0)):
                    bsl = slice(b * SB, (b + 1) * SB)
                    ntt = 4 * b + 4
                    for h in range(HPC):
                        pa = pavp.tile([128, SB], F32, tag="pav")
                        pn = psN.tile([1, SB], F32, tag="nrm")
                        psc = {}

                        def score(j):
                            psc[j] = psS.tile([128, SB], F32, tag="sc",
                                              name=f"sc{j}")
                            nc.tensor.matmul(
                                psc[j],
                                qe[HPC][:, j * 128:(j + 1) * 128],
                                qe[h][:, bsl],
                                start=True, stop=True)

                        def consume(j):
                            ex = p2e.tile([128, SB], BF16, tag="ex")
                            nc.scalar.activation(
                                ex, psc.pop(j),
                                mybir.ActivationFunctionType.Exp)
                            if j >= 4 * b:
                                nc.gpsimd.tensor_mul(
                                    ex, ex, mt[:, j - 4 * b, :])
                            nc.tensor.matmul(
                                pa, vn[:, j, :], ex,
                                start=(j == 0), stop=(j == ntt - 1))
                            nc.tensor.matmul(
                                pn[:], on, ex,
                                start=(j == 0), stop=(j == ntt - 1))

                        score(0)
                        if ntt > 1:
                            score(1)
                        for j in range(ntt):
                            if j + 2 < ntt:
                                score(j + 2)
                            consume(j)
                        rc1 = p2t.tile([1, SB], F32, tag="rc1")
                        nc.vector.reciprocal_approx_fast(rc1, pn)
                        rcp = p2t.tile([128, SB], F32, tag="rcp")
                        nc.gpsimd.partition_broadcast(rcp, rc1)
                        avn = p2a.tile([128, SB], BF16, tag="avn")
                        nc.vector.tensor_mul(avn, pa, rcp)
                        nc.sync.dma_start(
                            ag_in[bi][:][h * 128:(h + 1) * 128, :], avn)
                    nc.gpsimd.collective_compute(
                        "AllGather",
                        mybir.AluOpType.bypass,
                        ins=[ag_in[bi].opt()],
                        outs=[ag_out[bi].opt()],
                        replica_groups=RG,
                    )
                    if bi >= 1:
                        outproj(bi - 1)
                outproj(NSB - 1)
    nc.compile()
    return nc


_CACHE = {}


def _get_program():
    if "nc" not in _CACHE:
        _CACHE["nc"] = build()
    return _CACHE["nc"]


def _host_prep(x, freqs_cos, freqs_sin, wq, wk, wv, wo):
    x2 = np.ascontiguousarray(np.asarray(x, np.float32).reshape(S, D))
    xT = np.ascontiguousarray(x2.T).astype(NPBF)
    # even|odd -> [evens;odds] row permutation per head (RoPE partition split)
    perm1 = np.concatenate([np.arange(0, HD, 2), np.arange(1, HD, 2)])
    permq = (np.arange(H)[:, None] * HD + perm1[None, :]).reshape(-1)
    permk = (np.arange(KV)[:, None] * HD + perm1[None, :]).reshape(-1)
    scale = np.float32(1.0 / np.sqrt(HD))
    wq_p = np.asarray(wq, np.float32)[permq] * scale
    wk_p = np.asarray(wk, np.float32)[permk]
    wv32 = np.asarray(wv, np.float32)
    wo32 = np.asarray(wo, np.float32)
    cosT = np.asarray(freqs_cos, np.float32).T
    sinT = np.asarray(freqs_sin, np.float32).T
    ccb = np.ascontiguousarray(np.concatenate([cosT, cosT], 0))
    ssb = np.ascontiguousarray(np.concatenate([sinT, -sinT], 0))
    tp = np.arange(128, dtype=np.int64)[:, None]
    sf = np.arange(SB, dtype=np.int64)[None, :]
    masks = np.stack(
        [(sf >= tp + 128 * m).astype(NPBF) for m in range(HPC)], 0)
    ones = np.ones((128, 1), NPBF)
    ident = np.eye(128, dtype=NPBF)

    in_maps = []
    for i in range(NCORES):
        wqkv = np.concatenate(
            [wq_p[i * EQ:(i + 1) * EQ],
             wk_p[i * HD:(i + 1) * HD],
             wv32[i * HD:(i + 1) * HD]], 0)
        wqkvt = np.ascontiguousarray(wqkv.T).astype(NPBF)
        w3t = np.ascontiguousarray(
            wo32[i * DSH:(i + 1) * DSH, :].T).astype(NPBF)  # [4096, 512]
        in_maps.append(dict(xt=xT, wqkvt=wqkvt, w3t=w3t, cc=ccb, ss=ssb,
                            masks=masks, ones=ones, ident=ident))
    return in_maps


def _run(in_maps, trace=False):
    nc = _get_program()
    return run_bass_kernel_spmd(
        nc, in_maps, core_ids=list(range(NCORES)), trace=trace)


def _assemble(res):
    full = np.empty((S, D), np.float32)
    for r in range(NCORES):
        shard = np.asarray(res.results[r]["out"]).astype(np.float32)
        full[:, r * DSH:(r + 1) * DSH] = shard
    return full.reshape(B, S, D)


def kernel(x, freqs_cos, freqs_sin, wq, wk, wv, wo):
    in_maps = _host_prep(x, freqs_cos, freqs_sin, wq, wk, wv, wo)
    res = _run(in_maps, trace=False)
    return _assemble(res)


def _build_sharded():
    """Mirror of bass2jax.run_bass_via_pjrt's multi-core path, split so the
    jitted callable and device-resident inputs can be reused for timing."""
    import jax
    from jax.experimental.shard_map import shard_map
    from jax.sharding import Mesh, PartitionSpec

    import concourse.mybir as mb
    from concourse import bass2jax

    nc = _get_program()
    bass2jax.install_neuronx_cc_hook()
    part_name = (nc.partition_id_tensor.name
                 if nc.partition_id_tensor else None)
    in_names, out_names, out_avals, zero_outs = [], [], [], []
    for alloc in nc.m.functions[0].allocations:
        if not isinstance(alloc, mb.MemoryLocationSet):
            continue
        name = alloc.memorylocations[0].name
        if alloc.kind == "ExternalInput":
            if name != part_name:
                in_names.append(name)
        elif alloc.kind == "ExternalOutput":
            out_names.append(name)
            shape = tuple(alloc.tensor_shape)
            dtype = mb.dt.np(alloc.dtype)
            out_avals.append(jax.core.ShapedArray(shape, dtype))
            zero_outs.append(np.zeros(shape, dtype))
    n_params = len(in_names)
    all_names = in_names + out_names
    if part_name is not None:
        all_names = all_names + [part_name]

    def _body(*args):
        operands = list(args)
        if part_name is not None:
            operands.append(bass2jax.partition_id_tensor())
        outs = bass2jax._bass_exec_p.bind(
            *operands,
            out_avals=tuple(out_avals),
            in_names=tuple(all_names),
            out_names=tuple(out_names),
            lowering_input_output_aliases=(),
            sim_require_finite=True,
            sim_require_nnan=True,
            nc=nc,
        )
        return tuple(outs)

    devices = jax.devices()[:NCORES]
    mesh = Mesh(np.asarray(devices), ("core",))
    n_outs = len(out_names)
    sharded = jax.jit(
        shard_map(
            _body, mesh=mesh,
            in_specs=(PartitionSpec("core"),) * (n_params + n_outs),
            out_specs=(PartitionSpec("core"),) * n_outs,
            check_rep=False,
        ),
        donate_argnums=tuple(range(n_params, n_params + n_outs)),
        keep_unused=True,
    )
    return sharded, in_names, out_names, out_avals, zero_outs, mesh


def kernel_profiled(x, freqs_cos, freqs_sin, wq, wk, wv, wo, iters=12):
    """Returns (output, per-execution wall ns). Times repeated on-device
    executions with inputs pre-placed on the devices."""
    import time

    import jax
    from jax.sharding import NamedSharding, PartitionSpec

    in_maps = _host_prep(x, freqs_cos, freqs_sin, wq, wk, wv, wo)
    sharded, in_names, out_names, out_avals, zero_outs, mesh = _build_sharded()
    spec = NamedSharding(mesh, PartitionSpec("core"))
    concat_in = [
        jax.device_put(
            np.concatenate([in_maps[c][n] for c in range(NCORES)], axis=0),
            spec)
        for n in in_names
    ]

    def zeros():
        return [
            jax.device_put(
                np.zeros((NCORES * z.shape[0], *z.shape[1:]), z.dtype), spec)
            for z in zero_outs
        ]

    out_arrs = sharded(*concat_in, *zeros())  # warmup & result
    jax.block_until_ready(out_arrs)
    result = [np.asarray(a) for a in out_arrs]

    zsets = [zeros() for _ in range(iters)]
    jax.block_until_ready(zsets)
    t0 = time.perf_counter()
    last = None
    for zs in zsets:
        last = sharded(*concat_in, *zs)
    jax.block_until_ready(last)
    t1 = time.perf_counter()
    per_iter_ns = (t1 - t0) / iters * 1e9

    res_maps = [
        {n: result[i].reshape(NCORES, *out_avals[i].shape)[c]
         for i, n in enumerate(out_names)}
        for c in range(NCORES)
    ]

    class _R:
        results = res_maps

    return _assemble(_R), per_iter_ns


def _enable_ntff_hook():
    """Synthesize antenv.axon_hooks (absent in this image) and register the
    ctypes NTFF profile hook so run_bass_kernel_spmd(trace=True) works."""
    import sys as _sys
    import types as _types

    if "antenv.axon_hooks" in _sys.modules:
        return
    import antenv  # noqa: F401
    mod = _types.ModuleType("antenv.axon_hooks")
    mod._hook = None

    def set_axon_ntff_profile_hook(h):
        mod._hook = h

    def get_axon_ntff_profile_hook():
        return mod._hook

    mod.set_axon_ntff_profile_hook = set_axon_ntff_profile_hook
    mod.get_axon_ntff_profile_hook = get_axon_ntff_profile_hook
    _sys.modules["antenv.axon_hooks"] = mod
    antenv.axon_hooks = mod
    from trn_agent_boot.trn_boot import _ntff_profile_via_ctypes
    hook = _ntff_profile_via_ctypes("/opt/axon/libaxon_pjrt.so")
    if hook is not None:
        mod.set_axon_ntff_profile_hook(hook)
    # uploads need a fish bucket this container lacks; neuter them
    import concourse.bass_utils as _bu
    _bu.upload_artifacts = lambda tmpdir: f"local:{tmpdir}"


def kernel_traced(x, freqs_cos, freqs_sin, wq, wk, wv, wo, tmpdir=None):
    """Run once with NTFF tracing; returns (output, BassKernelResults)."""
    _enable_ntff_hook()
    in_maps = _host_prep(x, freqs_cos, freqs_sin, wq, wk, wv, wo)
    nc = _get_program()
    res = run_bass_kernel_spmd(
        nc, in_maps, core_ids=list(range(NCORES)), trace=True, tmpdir=tmpdir)
    return _assemble(res), res


# revision 12
# speedup vs baseline: 1.2625x; 1.2625x over previous
"""GQA causal attention (B=1, S=2048, D=4096, H=32, KV=8) on 8 trn2 cores.

Strategy: tensor-parallel over heads for QKV+attention, tensor-parallel
over output columns for the out-projection. Core i owns q-heads 4i..4i+3
and kv-head i. Host pre-transposes weights/x so every matmul contracts
along the partition dim, and pre-permutes wq/wk rows (even|odd
interleave -> [evens;odds]) so RoPE becomes partition-aligned
elementwise math.

Phase 1 (QKV+RoPE) streams x do-tile-major with six concurrent PSUM
accumulation groups so compute starts as soon as the first 128-row
chunk of x/w lands. Attention runs head-locally in a scores^T [t, s]
layout with a two-deep score-matmul pipeline ahead of the exp/mask
chain (exp on Scalar, causal mask multiply on GpSimd, softmax
normalization sums from a ones-vector matmul). Each block's normalized
head outputs are AllGathered (bf16); every core then computes its
512-column shard of the output projection for those tokens using all
32 heads, overlapped so the AllGather of block k hides under the
attention of block k+1 and the out-projection of block k-1. Host
concatenates column shards.

Matmul operands are bf16; accumulation, softmax and RoPE math are fp32.
"""

import sys

import numpy as np

sys.path.insert(0, "/opt/trn_rl_repo")

import ml_dtypes  # noqa: E402

import concourse.bass as bass  # noqa: E402
from concourse import bacc  # noqa: E402
import concourse.mybir as mybir  # noqa: E402
import concourse.tile as tile  # noqa: E402
from concourse.bass_utils import run_bass_kernel_spmd  # noqa: E402

F32 = mybir.dt.float32
BF16 = mybir.dt.bfloat16
NPBF = ml_dtypes.bfloat16

B, S, D = 1, 2048, 4096
H, KV, HD = 32, 8, 128
NCORES = 8
HPC = H // NCORES  # q heads per core = 4
EQ = HPC * HD  # 512 local q features
NE = HPC + 2  # e-tiles per core: 4 q + 1 k + 1 v
SB = 512  # attention s block
NSB = S // SB  # 4
XSB = 512  # phase-1 s sub-block (matmul moving dim)
DO = D // 128  # 32 contraction tiles for qkv projection
EO = (H * HD) // 128  # 32 contraction tiles for out-proj
DSH = D // NCORES  # 512 out-proj columns per core
TT = S // 128  # 16 t-tiles
DC = 4  # phase-1 do-tiles per DMA chunk
RG = [list(range(NCORES))]


def build():
    nc = bacc.Bacc("TRN2", target_bir_lowering=False)
    xt = nc.dram_tensor("xt", [D, S], BF16, kind="ExternalInput")
    wqkvt = nc.dram_tensor("wqkvt", [D, NE * 128], BF16, kind="ExternalInput")
    w3t = nc.dram_tensor("w3t", [H * HD, DSH], BF16, kind="ExternalInput")
    cc = nc.dram_tensor("cc", [128, S], F32, kind="ExternalInput")
    ss = nc.dram_tensor("ss", [128, S], F32, kind="ExternalInput")
    masks = nc.dram_tensor("masks", [HPC, 128, SB], BF16,
                           kind="ExternalInput")
    ones = nc.dram_tensor("ones", [128, 1], BF16, kind="ExternalInput")
    ident = nc.dram_tensor("ident", [128, 128], BF16, kind="ExternalInput")
    out = nc.dram_tensor("out", [S, DSH], BF16, kind="ExternalOutput")

    xt_t = xt[:].rearrange("(do p) s -> p do s", p=128)
    w_t = wqkvt[:].rearrange("(do p) e -> p do e", p=128)
    w3_t = w3t[:].rearrange("(eo p) d -> p eo d", p=128)

    with tile.TileContext(nc) as tc:
        with tc.tile_pool(name="dram", bufs=1, space="DRAM") as dram, \
                tc.tile_pool(name="pqkv", bufs=1) as pqkv:
            ag_in = [dram.tile([EQ, SB], BF16, name=f"agi{bi}")
                     for bi in range(NSB)]
            ag_out = [dram.tile([H * HD, SB], BF16, name=f"ago{bi}")
                      for bi in range(NSB)]
            qe = [pqkv.tile([128, S], BF16, name=f"qe{et}", tag=f"qe{et}")
                  for et in range(NE)]
            # v in natural [t, hd] tiles (transposed from e-tile layout)
            vn = pqkv.tile([128, TT, HD], BF16)
            idt = pqkv.tile([128, 128], BF16)
            nc.sync.dma_start(idt, ident[:])

            # ---------------- Phase 1: fused QKV projection + RoPE ----------
            with tc.tile_pool(name="p1w", bufs=1) as p1w, \
                    tc.tile_pool(name="p1x", bufs=2) as p1x, \
                    tc.tile_pool(name="p1t", bufs=2) as p1t, \
                    tc.tile_pool(name="p1ps", bufs=1, space="PSUM") as p1ps, \
                    tc.tile_pool(name="p1pt", bufs=2, space="PSUM") as p1pt:
                w = p1w.tile([128, DO, NE * 128], BF16)
                xtiles = []
                # stream first two x blocks and weights in do-chunks so the
                # PE can start on do 0..3 while the rest is in flight; x on
                # the sync queue, w on the scalar queue (parallel arrival)
                for sb in range(2):
                    ssl = slice(sb * XSB, (sb + 1) * XSB)
                    xtile = p1x.tile([128, DO, XSB], BF16, tag="x")
                    for dcb in range(DO // DC):
                        dsl = slice(dcb * DC, (dcb + 1) * DC)
                        nc.sync.dma_start(xtile[:, dsl, :], xt_t[:, dsl, ssl])
                        if sb == 0:
                            nc.scalar.dma_start(w[:, dsl, :], w_t[:, dsl, :])
                    xtiles.append(xtile)
                for sb in range(NSB):
                    ssl = slice(sb * XSB, (sb + 1) * XSB)
                    if sb < 2:
                        xtile = xtiles[sb]
                    else:
                        xtile = p1x.tile([128, DO, XSB], BF16, tag="x")
                        nc.sync.dma_start(xtile, xt_t[:, :, ssl])
                    cct = p1x.tile([128, XSB], F32, tag="cc")
                    sst = p1x.tile([128, XSB], F32, tag="ss")
                    nc.sync.dma_start(cct, cc[:][:, ssl])
                    nc.sync.dma_start(sst, ss[:][:, ssl])
                    atile = p1t.tile([128, HPC + 1, XSB], F32, tag="at",
                                     bufs=1)
                    btile = p1t.tile([128, HPC + 1, XSB], F32, tag="bt",
                                     bufs=1)
                    bsw = p1t.tile([128, (HPC + 1) * XSB], F32, tag="bsw",
                                   bufs=1)
                    ps = [p1ps.tile([128, XSB], F32, tag=f"ps{et}",
                                    name=f"ps{et}")
                          for et in range(NE)]
                    # do-major: six concurrent PSUM groups; MMs for do-chunk
                    # d need only chunk d of x and w
                    for do in range(DO):
                        for et in range(NE):
                            nc.tensor.matmul(
                                ps[et],
                                w[:, do, et * 128:(et + 1) * 128],
                                xtile[:, do, :],
                                start=(do == 0),
                                stop=(do == DO - 1),
                            )
                    for et in range(NE):
                        if et < HPC + 1:
                            # t*cos and t*sin halves; swap+add finishes RoPE
                            nc.vector.tensor_mul(atile[:, et, :], ps[et], cct)
                            nc.vector.tensor_mul(btile[:, et, :], ps[et], sst)
                        else:
                            nc.any.tensor_copy(qe[et][:, ssl], ps[et])
                    # transpose this s-block's four v t-tiles right away
                    for tt in range(sb * 4, sb * 4 + 4):
                        pst = p1pt.tile([128, 128], BF16, tag="pst")
                        nc.tensor.transpose(
                            pst, qe[HPC + 1][:, tt * 128:(tt + 1) * 128], idt)
                        nc.any.tensor_copy(vn[:, tt, :], pst)
                    bt2 = btile.rearrange("p e s -> p (e s)")
                    nc.sync.dma_start(bsw[:64, :], bt2[64:, :])
                    nc.sync.dma_start(bsw[64:, :], bt2[:64, :])
                    bs3 = bsw.rearrange("p (e s) -> p e s", s=XSB)
                    for et in range(HPC + 1):
                        nc.vector.tensor_add(
                            qe[et][:, ssl], atile[:, et, :], bs3[:, et, :])

            # ------- Phase 2+3: attention blocks + pipelined out-proj -------
            with tc.tile_pool(name="p2c", bufs=1) as p2c, \
                    tc.tile_pool(name="p2e", bufs=4) as p2e, \
                    tc.tile_pool(name="p2t", bufs=2) as p2t, \
                    tc.tile_pool(name="p2a", bufs=4) as p2a, \
                    tc.tile_pool(name="p3a", bufs=2) as p3a, \
                    tc.tile_pool(name="p3o", bufs=2) as p3o, \
                    tc.tile_pool(name="psS", bufs=2, space="PSUM") as psS, \
                    tc.tile_pool(name="pav", bufs=2, space="PSUM") as pavp, \
                    tc.tile_pool(name="psN", bufs=2, space="PSUM") as psN, \
                    tc.tile_pool(name="pop", bufs=2, space="PSUM") as popp:
                # tiny setup tensors first (sync queue) so the vn transposes
                # aren't stuck behind the big w3 stream (scalar queue)
                mt = p2c.tile([128, HPC, SB], BF16)
                nc.sync.dma_start(mt, masks[:].rearrange("m p s -> p m s"))
                on = p2c.tile([128, 1], BF16)
                nc.sync.dma_start(on, ones[:])
                w3sb = p2c.tile([128, EO, DSH], BF16)
                nc.scalar.dma_start(w3sb, w3_t)

                def outproj(bi):
                    bb = 3 - bi  # block id of buffer bi
                    aog = p3a.tile([128, EO, SB], BF16, tag="aog")
                    ag_r = ag_out[bi][:].rearrange("(eo p) s -> p eo s",
                                                   p=128)
                    # halves, so the first s-tile's matmuls start after ~2MB
                    for hf in range(2):
                        hsl = slice(hf * (SB // 2), (hf + 1) * (SB // 2))
                        nc.sync.dma_start(aog[:, :, hsl], ag_r[:, :, hsl])
                    for stl in range(4):
                        st = bb * 4 + stl
                        po = popp.tile([128, DSH], F32, tag="pop")
                        for eo in range(EO):
                            nc.tensor.matmul(
                                po,
                                aog[:, eo, stl * 128:(stl + 1) * 128],
                                w3sb[:, eo, :],
                                start=(eo == 0), stop=(eo == EO - 1))
                        osb = p3o.tile([128, DSH], BF16, tag="osb")
                        nc.vector.tensor_copy(osb, po)
                        nc.sync.dma_start(
                            out[:][st * 128:(st + 1) * 128, :], osb)

                for bi, b in enumerate((3, 2, 1, 0)):
                    bsl = slice(b * SB, (b + 1) * SB)
                    ntt = 4 * b + 4
                    for h in range(HPC):
                        pa = pavp.tile([128, SB], F32, tag="pav")
                        pn = psN.tile([1, SB], F32, tag="nrm")
                        psc = {}

                        def score(j):
                            psc[j] = psS.tile([128, SB], F32, tag="sc",
                                              name=f"sc{j}")
                            nc.tensor.matmul(
                                psc[j],
                                qe[HPC][:, j * 128:(j + 1) * 128],
                                qe[h][:, bsl],
                                start=True, stop=True)

                        def consume(j):
                            ex = p2e.tile([128, SB], BF16, tag="ex")
                            nc.scalar.activation(
                                ex, psc.pop(j),
                                mybir.ActivationFunctionType.Exp)
                            if j >= 4 * b:
                                nc.vector.tensor_mul(
                                    ex, ex, mt[:, j - 4 * b, :])
                            nc.tensor.matmul(
                                pa, vn[:, j, :], ex,
                                start=(j == 0), stop=(j == ntt - 1))
                            nc.tensor.matmul(
                                pn[:], on, ex,
                                start=(j == 0), stop=(j == ntt - 1))

                        score(0)
                        if ntt > 1:
                            score(1)
                        for j in range(ntt):
                            if j + 2 < ntt:
                                score(j + 2)
                            consume(j)
                        rc1 = p2t.tile([1, SB], F32, tag="rc1")
                        nc.vector.reciprocal_approx_fast(rc1, pn)
                        rcp = p2t.tile([128, SB], F32, tag="rcp")
                        nc.gpsimd.partition_broadcast(rcp, rc1)
                        avn = p2a.tile([128, SB], BF16, tag="avn")
                        nc.vector.tensor_mul(avn, pa, rcp)
                        nc.sync.dma_start(
                            ag_in[bi][:][h * 128:(h + 1) * 128, :], avn)
                    nc.gpsimd.collective_compute(
                        "AllGather",
                        mybir.AluOpType.bypass,
                        ins=[ag_in[bi].opt()],
                        outs=[ag_out[bi].opt()],
                        replica_groups=RG,
                    )
                    if bi >= 1:
                        outproj(bi - 1)
                outproj(NSB - 1)
    nc.compile()
    return nc


_CACHE = {}


def _get_program():
    if "nc" not in _CACHE:
        _CACHE["nc"] = build()
    return _CACHE["nc"]


def _host_prep(x, freqs_cos, freqs_sin, wq, wk, wv, wo):
    x2 = np.ascontiguousarray(np.asarray(x, np.float32).reshape(S, D))
    xT = np.ascontiguousarray(x2.T).astype(NPBF)
    # even|odd -> [evens;odds] row permutation per head (RoPE partition split)
    perm1 = np.concatenate([np.arange(0, HD, 2), np.arange(1, HD, 2)])
    permq = (np.arange(H)[:, None] * HD + perm1[None, :]).reshape(-1)
    permk = (np.arange(KV)[:, None] * HD + perm1[None, :]).reshape(-1)
    scale = np.float32(1.0 / np.sqrt(HD))
    wq_p = np.asarray(wq, np.float32)[permq] * scale
    wk_p = np.asarray(wk, np.float32)[permk]
    wv32 = np.asarray(wv, np.float32)
    wo32 = np.asarray(wo, np.float32)
    cosT = np.asarray(freqs_cos, np.float32).T
    sinT = np.asarray(freqs_sin, np.float32).T
    ccb = np.ascontiguousarray(np.concatenate([cosT, cosT], 0))
    ssb = np.ascontiguousarray(np.concatenate([sinT, -sinT], 0))
    tp = np.arange(128, dtype=np.int64)[:, None]
    sf = np.arange(SB, dtype=np.int64)[None, :]
    masks = np.stack(
        [(sf >= tp + 128 * m).astype(NPBF) for m in range(HPC)], 0)
    ones = np.ones((128, 1), NPBF)
    ident = np.eye(128, dtype=NPBF)

    in_maps = []
    for i in range(NCORES):
        wqkv = np.concatenate(
            [wq_p[i * EQ:(i + 1) * EQ],
             wk_p[i * HD:(i + 1) * HD],
             wv32[i * HD:(i + 1) * HD]], 0)
        wqkvt = np.ascontiguousarray(wqkv.T).astype(NPBF)
        w3t = np.ascontiguousarray(
            wo32[i * DSH:(i + 1) * DSH, :].T).astype(NPBF)  # [4096, 512]
        in_maps.append(dict(xt=xT, wqkvt=wqkvt, w3t=w3t, cc=ccb, ss=ssb,
                            masks=masks, ones=ones, ident=ident))
    return in_maps


def _run(in_maps, trace=False):
    nc = _get_program()
    return run_bass_kernel_spmd(
        nc, in_maps, core_ids=list(range(NCORES)), trace=trace)


def _assemble(res):
    full = np.empty((S, D), np.float32)
    for r in range(NCORES):
        shard = np.asarray(res.results[r]["out"]).astype(np.float32)
        full[:, r * DSH:(r + 1) * DSH] = shard
    return full.reshape(B, S, D)


def kernel(x, freqs_cos, freqs_sin, wq, wk, wv, wo):
    in_maps = _host_prep(x, freqs_cos, freqs_sin, wq, wk, wv, wo)
    res = _run(in_maps, trace=False)
    return _assemble(res)


def _build_sharded():
    """Mirror of bass2jax.run_bass_via_pjrt's multi-core path, split so the
    jitted callable and device-resident inputs can be reused for timing."""
    import jax
    from jax.experimental.shard_map import shard_map
    from jax.sharding import Mesh, PartitionSpec

    import concourse.mybir as mb
    from concourse import bass2jax

    nc = _get_program()
    bass2jax.install_neuronx_cc_hook()
    part_name = (nc.partition_id_tensor.name
                 if nc.partition_id_tensor else None)
    in_names, out_names, out_avals, zero_outs = [], [], [], []
    for alloc in nc.m.functions[0].allocations:
        if not isinstance(alloc, mb.MemoryLocationSet):
            continue
        name = alloc.memorylocations[0].name
        if alloc.kind == "ExternalInput":
            if name != part_name:
                in_names.append(name)
        elif alloc.kind == "ExternalOutput":
            out_names.append(name)
            shape = tuple(alloc.tensor_shape)
            dtype = mb.dt.np(alloc.dtype)
            out_avals.append(jax.core.ShapedArray(shape, dtype))
            zero_outs.append(np.zeros(shape, dtype))
    n_params = len(in_names)
    all_names = in_names + out_names
    if part_name is not None:
        all_names = all_names + [part_name]

    def _body(*args):
        operands = list(args)
        if part_name is not None:
            operands.append(bass2jax.partition_id_tensor())
        outs = bass2jax._bass_exec_p.bind(
            *operands,
            out_avals=tuple(out_avals),
            in_names=tuple(all_names),
            out_names=tuple(out_names),
            lowering_input_output_aliases=(),
            sim_require_finite=True,
            sim_require_nnan=True,
            nc=nc,
        )
        return tuple(outs)

    devices = jax.devices()[:NCORES]
    mesh = Mesh(np.asarray(devices), ("core",))
    n_outs = len(out_names)
    sharded = jax.jit(
        shard_map(
            _body, mesh=mesh,
            in_specs=(PartitionSpec("core"),) * (n_params + n_outs),
            out_specs=(PartitionSpec("core"),) * n_outs,
            check_rep=False,
        ),
        donate_argnums=tuple(range(n_params, n_params + n_outs)),
        keep_unused=True,
    )
    return sharded, in_names, out_names, out_avals, zero_outs, mesh


def kernel_profiled(x, freqs_cos, freqs_sin, wq, wk, wv, wo, iters=12):
    """Returns (output, per-execution wall ns). Times repeated on-device
    executions with inputs pre-placed on the devices."""
    import time

    import jax
    from jax.sharding import NamedSharding, PartitionSpec

    in_maps = _host_prep(x, freqs_cos, freqs_sin, wq, wk, wv, wo)
    sharded, in_names, out_names, out_avals, zero_outs, mesh = _build_sharded()
    spec = NamedSharding(mesh, PartitionSpec("core"))
    concat_in = [
        jax.device_put(
            np.concatenate([in_maps[c][n] for c in range(NCORES)], axis=0),
            spec)
        for n in in_names
    ]

    def zeros():
        return [
            jax.device_put(
                np.zeros((NCORES * z.shape[0], *z.shape[1:]), z.dtype), spec)
            for z in zero_outs
        ]

    out_arrs = sharded(*concat_in, *zeros())  # warmup & result
    jax.block_until_ready(out_arrs)
    result = [np.asarray(a) for a in out_arrs]

    zsets = [zeros() for _ in range(iters)]
    jax.block_until_ready(zsets)
    t0 = time.perf_counter()
    last = None
    for zs in zsets:
        last = sharded(*concat_in, *zs)
    jax.block_until_ready(last)
    t1 = time.perf_counter()
    per_iter_ns = (t1 - t0) / iters * 1e9

    res_maps = [
        {n: result[i].reshape(NCORES, *out_avals[i].shape)[c]
         for i, n in enumerate(out_names)}
        for c in range(NCORES)
    ]

    class _R:
        results = res_maps

    return _assemble(_R), per_iter_ns


def _enable_ntff_hook():
    """Synthesize antenv.axon_hooks (absent in this image) and register the
    ctypes NTFF profile hook so run_bass_kernel_spmd(trace=True) works."""
    import sys as _sys
    import types as _types

    if "antenv.axon_hooks" in _sys.modules:
        return
    import antenv  # noqa: F401
    mod = _types.ModuleType("antenv.axon_hooks")
    mod._hook = None

    def set_axon_ntff_profile_hook(h):
        mod._hook = h

    def get_axon_ntff_profile_hook():
        return mod._hook

    mod.set_axon_ntff_profile_hook = set_axon_ntff_profile_hook
    mod.get_axon_ntff_profile_hook = get_axon_ntff_profile_hook
    _sys.modules["antenv.axon_hooks"] = mod
    antenv.axon_hooks = mod
    from trn_agent_boot.trn_boot import _ntff_profile_via_ctypes
    hook = _ntff_profile_via_ctypes("/opt/axon/libaxon_pjrt.so")
    if hook is not None:
        mod.set_axon_ntff_profile_hook(hook)
    # uploads need a fish bucket this container lacks; neuter them
    import concourse.bass_utils as _bu
    _bu.upload_artifacts = lambda tmpdir: f"local:{tmpdir}"


def kernel_traced(x, freqs_cos, freqs_sin, wq, wk, wv, wo, tmpdir=None):
    """Run once with NTFF tracing; returns (output, BassKernelResults)."""
    _enable_ntff_hook()
    in_maps = _host_prep(x, freqs_cos, freqs_sin, wq, wk, wv, wo)
    nc = _get_program()
    res = run_bass_kernel_spmd(
        nc, in_maps, core_ids=list(range(NCORES)), trace=True, tmpdir=tmpdir)
    return _assemble(res), res


# revision 15
# speedup vs baseline: 1.2956x; 1.0262x over previous
"""GQA causal attention (B=1, S=2048, D=4096, H=32, KV=8) on 8 trn2 cores.

Strategy: tensor-parallel over heads for QKV+attention, tensor-parallel
over output columns for the out-projection. Core i owns q-heads 4i..4i+3
and kv-head i. Host pre-transposes weights/x so every matmul contracts
along the partition dim, and pre-permutes wq/wk rows (even|odd
interleave -> [evens;odds]) so RoPE becomes partition-aligned
elementwise math.

Phase 1 (QKV+RoPE) streams x do-tile-major with six concurrent PSUM
accumulation groups so compute starts as soon as the first 128-row
chunk of x/w lands. Attention runs head-locally in a scores^T [t, s]
layout with a two-deep score-matmul pipeline ahead of the exp/mask
chain (exp on Scalar, causal mask multiply on GpSimd, softmax
normalization sums from a ones-vector matmul). Each block's normalized
head outputs are AllGathered (bf16); every core then computes its
512-column shard of the output projection for those tokens using all
32 heads, overlapped so the AllGather of block k hides under the
attention of block k+1 and the out-projection of block k-1. Host
concatenates column shards.

Matmul operands are bf16; accumulation, softmax and RoPE math are fp32.
"""

import sys

import numpy as np

sys.path.insert(0, "/opt/trn_rl_repo")

import ml_dtypes  # noqa: E402

import concourse.bass as bass  # noqa: E402
from concourse import bacc  # noqa: E402
import concourse.mybir as mybir  # noqa: E402
import concourse.tile as tile  # noqa: E402
from concourse.bass_utils import run_bass_kernel_spmd  # noqa: E402

F32 = mybir.dt.float32
BF16 = mybir.dt.bfloat16
NPBF = ml_dtypes.bfloat16

B, S, D = 1, 2048, 4096
H, KV, HD = 32, 8, 128
NCORES = 8
HPC = H // NCORES  # q heads per core = 4
EQ = HPC * HD  # 512 local q features
NE = HPC + 2  # e-tiles per core: 4 q + 1 k + 1 v
SB = 512  # attention s block
NSB = S // SB  # 4
XSB = 512  # phase-1 s sub-block (matmul moving dim)
DO = D // 128  # 32 contraction tiles for qkv projection
EO = (H * HD) // 128  # 32 contraction tiles for out-proj
DSH = D // NCORES  # 512 out-proj columns per core
TT = S // 128  # 16 t-tiles
DC = 4  # phase-1 do-tiles per DMA chunk
RG = [list(range(NCORES))]


def build():
    nc = bacc.Bacc("TRN2", target_bir_lowering=False)
    xt = nc.dram_tensor("xt", [D, S], BF16, kind="ExternalInput")
    wqkvt = nc.dram_tensor("wqkvt", [D, NE * 128], BF16, kind="ExternalInput")
    w3t = nc.dram_tensor("w3t", [H * HD, DSH], BF16, kind="ExternalInput")
    cc = nc.dram_tensor("cc", [128, S], F32, kind="ExternalInput")
    ss = nc.dram_tensor("ss", [128, S], F32, kind="ExternalInput")
    masks = nc.dram_tensor("masks", [HPC, 128, SB], BF16,
                           kind="ExternalInput")
    ones = nc.dram_tensor("ones", [128, 1], BF16, kind="ExternalInput")
    ident = nc.dram_tensor("ident", [128, 128], BF16, kind="ExternalInput")
    out = nc.dram_tensor("out", [S, DSH], BF16, kind="ExternalOutput")

    xt_t = xt[:].rearrange("(do p) s -> p do s", p=128)
    w_t = wqkvt[:].rearrange("(do p) e -> p do e", p=128)
    w3_t = w3t[:].rearrange("(eo p) d -> p eo d", p=128)

    with tile.TileContext(nc) as tc:
        with tc.tile_pool(name="dram", bufs=1, space="DRAM") as dram, \
                tc.tile_pool(name="pqkv", bufs=1) as pqkv:
            ag_in = [dram.tile([EQ, SB], BF16, name=f"agi{bi}")
                     for bi in range(NSB)]
            ag_out = [dram.tile([H * HD, SB], BF16, name=f"ago{bi}")
                      for bi in range(NSB)]
            qe = [pqkv.tile([128, S], BF16, name=f"qe{et}", tag=f"qe{et}")
                  for et in range(NE)]
            # v in natural [t, hd] tiles (transposed from e-tile layout)
            vn = pqkv.tile([128, TT, HD], BF16)
            idt = pqkv.tile([128, 128], BF16)
            nc.sync.dma_start(idt, ident[:])

            # ---------------- Phase 1: fused QKV projection + RoPE ----------
            with tc.tile_pool(name="p1w", bufs=1) as p1w, \
                    tc.tile_pool(name="p1x", bufs=2) as p1x, \
                    tc.tile_pool(name="p1t", bufs=2) as p1t, \
                    tc.tile_pool(name="p1ps", bufs=1, space="PSUM") as p1ps, \
                    tc.tile_pool(name="p1pt", bufs=2, space="PSUM") as p1pt:
                w = p1w.tile([128, DO, NE * 128], BF16)
                xtiles = []
                # stream first two x blocks and weights in do-chunks so the
                # PE can start on do 0..3 while the rest is in flight; x on
                # the sync queue, w on the scalar queue (parallel arrival)
                for sb in range(2):
                    ssl = slice(sb * XSB, (sb + 1) * XSB)
                    xtile = p1x.tile([128, DO, XSB], BF16, tag="x")
                    # finer chunks up front so the first matmuls start sooner
                    bounds = [0, 2, 4, 8, 12, 16, 20, 24, 28, 32]
                    for lo, hi in zip(bounds, bounds[1:]):
                        dsl = slice(lo, hi)
                        nc.sync.dma_start(xtile[:, dsl, :], xt_t[:, dsl, ssl])
                        if sb == 0:
                            nc.scalar.dma_start(w[:, dsl, :], w_t[:, dsl, :])
                    xtiles.append(xtile)
                for sb in range(NSB):
                    ssl = slice(sb * XSB, (sb + 1) * XSB)
                    if sb < 2:
                        xtile = xtiles[sb]
                    else:
                        xtile = p1x.tile([128, DO, XSB], BF16, tag="x")
                        nc.sync.dma_start(xtile, xt_t[:, :, ssl])
                    cct = p1x.tile([128, XSB], F32, tag="cc")
                    sst = p1x.tile([128, XSB], F32, tag="ss")
                    nc.sync.dma_start(cct, cc[:][:, ssl])
                    nc.sync.dma_start(sst, ss[:][:, ssl])
                    atile = p1t.tile([128, HPC + 1, XSB], F32, tag="at",
                                     bufs=1)
                    btile = p1t.tile([128, HPC + 1, XSB], F32, tag="bt",
                                     bufs=1)
                    bsw = p1t.tile([128, (HPC + 1) * XSB], F32, tag="bsw",
                                   bufs=1)
                    ps = [p1ps.tile([128, XSB], F32, tag=f"ps{et}",
                                    name=f"ps{et}")
                          for et in range(NE)]
                    # do-major: six concurrent PSUM groups; MMs for do-chunk
                    # d need only chunk d of x and w
                    for do in range(DO):
                        for et in range(NE):
                            nc.tensor.matmul(
                                ps[et],
                                w[:, do, et * 128:(et + 1) * 128],
                                xtile[:, do, :],
                                start=(do == 0),
                                stop=(do == DO - 1),
                            )
                    for et in range(NE):
                        if et < HPC + 1:
                            # t*cos and t*sin halves; swap+add finishes RoPE
                            nc.vector.tensor_mul(atile[:, et, :], ps[et], cct)
                            nc.vector.tensor_mul(btile[:, et, :], ps[et], sst)
                        else:
                            nc.any.tensor_copy(qe[et][:, ssl], ps[et])
                    # transpose this s-block's four v t-tiles right away
                    for tt in range(sb * 4, sb * 4 + 4):
                        pst = p1pt.tile([128, 128], BF16, tag="pst")
                        nc.tensor.transpose(
                            pst, qe[HPC + 1][:, tt * 128:(tt + 1) * 128], idt)
                        nc.any.tensor_copy(vn[:, tt, :], pst)
                    bt2 = btile.rearrange("p e s -> p (e s)")
                    nc.sync.dma_start(bsw[:64, :], bt2[64:, :])
                    nc.sync.dma_start(bsw[64:, :], bt2[:64, :])
                    bs3 = bsw.rearrange("p (e s) -> p e s", s=XSB)
                    for et in range(HPC + 1):
                        nc.vector.tensor_add(
                            qe[et][:, ssl], atile[:, et, :], bs3[:, et, :])

            # ------- Phase 2+3: attention blocks + pipelined out-proj -------
            with tc.tile_pool(name="p2c", bufs=1) as p2c, \
                    tc.tile_pool(name="p2e", bufs=4) as p2e, \
                    tc.tile_pool(name="p2t", bufs=2) as p2t, \
                    tc.tile_pool(name="p2a", bufs=4) as p2a, \
                    tc.tile_pool(name="p3a", bufs=2) as p3a, \
                    tc.tile_pool(name="p3o", bufs=2) as p3o, \
                    tc.tile_pool(name="psS", bufs=2, space="PSUM") as psS, \
                    tc.tile_pool(name="pav", bufs=2, space="PSUM") as pavp, \
                    tc.tile_pool(name="psN", bufs=2, space="PSUM") as psN, \
                    tc.tile_pool(name="pop", bufs=2, space="PSUM") as popp:
                # tiny setup tensors first (sync queue) so the vn transposes
                # aren't stuck behind the big w3 stream (scalar queue)
                mt = p2c.tile([128, HPC, SB], BF16)
                nc.sync.dma_start(mt, masks[:].rearrange("m p s -> p m s"))
                on = p2c.tile([128, 1], BF16)
                nc.sync.dma_start(on, ones[:])
                w3sb = p2c.tile([128, EO, DSH], BF16)
                nc.scalar.dma_start(w3sb, w3_t)

                def outproj(bb):
                    aog = p3a.tile([128, EO, SB], BF16, tag="aog")
                    ag_r = ag_out[bb][:].rearrange("(eo p) s -> p eo s",
                                                   p=128)
                    # halves, so the first s-tile's matmuls start after ~2MB
                    for hf in range(2):
                        hsl = slice(hf * (SB // 2), (hf + 1) * (SB // 2))
                        nc.sync.dma_start(aog[:, :, hsl], ag_r[:, :, hsl])
                    for stl in range(4):
                        st = bb * 4 + stl
                        po = popp.tile([128, DSH], F32, tag="pop")
                        for eo in range(EO):
                            nc.tensor.matmul(
                                po,
                                aog[:, eo, stl * 128:(stl + 1) * 128],
                                w3sb[:, eo, :],
                                start=(eo == 0), stop=(eo == EO - 1))
                        osb = p3o.tile([128, DSH], BF16, tag="osb")
                        nc.vector.tensor_copy(osb, po)
                        nc.sync.dma_start(
                            out[:][st * 128:(st + 1) * 128, :], osb)

                for b in range(NSB):
                    bsl = slice(b * SB, (b + 1) * SB)
                    ntt = 4 * b + 4
                    for h in range(HPC):
                        pa = pavp.tile([128, SB], F32, tag="pav")
                        pn = psN.tile([1, SB], F32, tag="nrm")
                        psc = {}

                        def score(j):
                            # diagonal tile m only contributes s >= 128m
                            off = max(0, (j - 4 * b) * 128)
                            psc[j] = psS.tile([128, SB], F32, tag="sc",
                                              name=f"sc{j}")
                            nc.tensor.matmul(
                                psc[j][:, off:],
                                qe[HPC][:, j * 128:(j + 1) * 128],
                                qe[h][:, b * SB + off:(b + 1) * SB],
                                start=True, stop=True)

                        def consume(j):
                            off = max(0, (j - 4 * b) * 128)
                            ex = p2e.tile([128, SB], BF16, tag="ex")
                            nc.scalar.activation(
                                ex[:, off:], psc.pop(j)[:, off:],
                                mybir.ActivationFunctionType.Exp)
                            if j >= 4 * b:
                                m = j - 4 * b
                                nc.vector.tensor_mul(
                                    ex[:, off:], ex[:, off:],
                                    mt[:, m, off:])
                            nc.tensor.matmul(
                                pa[:, off:], vn[:, j, :], ex[:, off:],
                                start=(j == 0), stop=(j == ntt - 1))
                            nc.tensor.matmul(
                                pn[:, off:], on, ex[:, off:],
                                start=(j == 0), stop=(j == ntt - 1))

                        score(0)
                        if ntt > 1:
                            score(1)
                        for j in range(ntt):
                            if j + 2 < ntt:
                                score(j + 2)
                            consume(j)
                        rc1 = p2t.tile([1, SB], F32, tag="rc1")
                        nc.vector.reciprocal_approx_fast(rc1, pn)
                        rcp = p2t.tile([128, SB], F32, tag="rcp")
                        nc.gpsimd.partition_broadcast(rcp, rc1)
                        avn = p2a.tile([128, SB], BF16, tag="avn")
                        nc.vector.tensor_mul(avn, pa, rcp)
                        nc.sync.dma_start(
                            ag_in[b][:][h * 128:(h + 1) * 128, :], avn)
                    nc.gpsimd.collective_compute(
                        "AllGather",
                        mybir.AluOpType.bypass,
                        ins=[ag_in[b].opt()],
                        outs=[ag_out[b].opt()],
                        replica_groups=RG,
                    )
                for b in range(NSB):
                    outproj(b)
    nc.compile()
    return nc


_CACHE = {}


def _get_program():
    if "nc" not in _CACHE:
        _CACHE["nc"] = build()
    return _CACHE["nc"]


def _host_prep(x, freqs_cos, freqs_sin, wq, wk, wv, wo):
    x2 = np.ascontiguousarray(np.asarray(x, np.float32).reshape(S, D))
    xT = np.ascontiguousarray(x2.T).astype(NPBF)
    # even|odd -> [evens;odds] row permutation per head (RoPE partition split)
    perm1 = np.concatenate([np.arange(0, HD, 2), np.arange(1, HD, 2)])
    permq = (np.arange(H)[:, None] * HD + perm1[None, :]).reshape(-1)
    permk = (np.arange(KV)[:, None] * HD + perm1[None, :]).reshape(-1)
    scale = np.float32(1.0 / np.sqrt(HD))
    wq_p = np.asarray(wq, np.float32)[permq] * scale
    wk_p = np.asarray(wk, np.float32)[permk]
    wv32 = np.asarray(wv, np.float32)
    wo32 = np.asarray(wo, np.float32)
    cosT = np.asarray(freqs_cos, np.float32).T
    sinT = np.asarray(freqs_sin, np.float32).T
    ccb = np.ascontiguousarray(np.concatenate([cosT, cosT], 0))
    ssb = np.ascontiguousarray(np.concatenate([sinT, -sinT], 0))
    tp = np.arange(128, dtype=np.int64)[:, None]
    sf = np.arange(SB, dtype=np.int64)[None, :]
    masks = np.stack(
        [(sf >= tp + 128 * m).astype(NPBF) for m in range(HPC)], 0)
    ones = np.ones((128, 1), NPBF)
    ident = np.eye(128, dtype=NPBF)

    in_maps = []
    for i in range(NCORES):
        wqkv = np.concatenate(
            [wq_p[i * EQ:(i + 1) * EQ],
             wk_p[i * HD:(i + 1) * HD],
             wv32[i * HD:(i + 1) * HD]], 0)
        wqkvt = np.ascontiguousarray(wqkv.T).astype(NPBF)
        w3t = np.ascontiguousarray(
            wo32[i * DSH:(i + 1) * DSH, :].T).astype(NPBF)  # [4096, 512]
        in_maps.append(dict(xt=xT, wqkvt=wqkvt, w3t=w3t, cc=ccb, ss=ssb,
                            masks=masks, ones=ones, ident=ident))
    return in_maps


def _run(in_maps, trace=False):
    nc = _get_program()
    return run_bass_kernel_spmd(
        nc, in_maps, core_ids=list(range(NCORES)), trace=trace)


def _assemble(res):
    full = np.empty((S, D), np.float32)
    for r in range(NCORES):
        shard = np.asarray(res.results[r]["out"]).astype(np.float32)
        full[:, r * DSH:(r + 1) * DSH] = shard
    return full.reshape(B, S, D)


def kernel(x, freqs_cos, freqs_sin, wq, wk, wv, wo):
    in_maps = _host_prep(x, freqs_cos, freqs_sin, wq, wk, wv, wo)
    res = _run(in_maps, trace=False)
    return _assemble(res)


def _build_sharded():
    """Mirror of bass2jax.run_bass_via_pjrt's multi-core path, split so the
    jitted callable and device-resident inputs can be reused for timing."""
    import jax
    from jax.experimental.shard_map import shard_map
    from jax.sharding import Mesh, PartitionSpec

    import concourse.mybir as mb
    from concourse import bass2jax

    nc = _get_program()
    bass2jax.install_neuronx_cc_hook()
    part_name = (nc.partition_id_tensor.name
                 if nc.partition_id_tensor else None)
    in_names, out_names, out_avals, zero_outs = [], [], [], []
    for alloc in nc.m.functions[0].allocations:
        if not isinstance(alloc, mb.MemoryLocationSet):
            continue
        name = alloc.memorylocations[0].name
        if alloc.kind == "ExternalInput":
            if name != part_name:
                in_names.append(name)
        elif alloc.kind == "ExternalOutput":
            out_names.append(name)
            shape = tuple(alloc.tensor_shape)
            dtype = mb.dt.np(alloc.dtype)
            out_avals.append(jax.core.ShapedArray(shape, dtype))
            zero_outs.append(np.zeros(shape, dtype))
    n_params = len(in_names)
    all_names = in_names + out_names
    if part_name is not None:
        all_names = all_names + [part_name]

    def _body(*args):
        operands = list(args)
        if part_name is not None:
            operands.append(bass2jax.partition_id_tensor())
        outs = bass2jax._bass_exec_p.bind(
            *operands,
            out_avals=tuple(out_avals),
            in_names=tuple(all_names),
            out_names=tuple(out_names),
            lowering_input_output_aliases=(),
            sim_require_finite=True,
            sim_require_nnan=True,
            nc=nc,
        )
        return tuple(outs)

    devices = jax.devices()[:NCORES]
    mesh = Mesh(np.asarray(devices), ("core",))
    n_outs = len(out_names)
    sharded = jax.jit(
        shard_map(
            _body, mesh=mesh,
            in_specs=(PartitionSpec("core"),) * (n_params + n_outs),
            out_specs=(PartitionSpec("core"),) * n_outs,
            check_rep=False,
        ),
        donate_argnums=tuple(range(n_params, n_params + n_outs)),
        keep_unused=True,
    )
    return sharded, in_names, out_names, out_avals, zero_outs, mesh


def kernel_profiled(x, freqs_cos, freqs_sin, wq, wk, wv, wo, iters=12):
    """Returns (output, per-execution wall ns). Times repeated on-device
    executions with inputs pre-placed on the devices."""
    import time

    import jax
    from jax.sharding import NamedSharding, PartitionSpec

    in_maps = _host_prep(x, freqs_cos, freqs_sin, wq, wk, wv, wo)
    sharded, in_names, out_names, out_avals, zero_outs, mesh = _build_sharded()
    spec = NamedSharding(mesh, PartitionSpec("core"))
    concat_in = [
        jax.device_put(
            np.concatenate([in_maps[c][n] for c in range(NCORES)], axis=0),
            spec)
        for n in in_names
    ]

    def zeros():
        return [
            jax.device_put(
                np.zeros((NCORES * z.shape[0], *z.shape[1:]), z.dtype), spec)
            for z in zero_outs
        ]

    out_arrs = sharded(*concat_in, *zeros())  # warmup & result
    jax.block_until_ready(out_arrs)
    result = [np.asarray(a) for a in out_arrs]

    zsets = [zeros() for _ in range(iters)]
    jax.block_until_ready(zsets)
    t0 = time.perf_counter()
    last = None
    for zs in zsets:
        last = sharded(*concat_in, *zs)
    jax.block_until_ready(last)
    t1 = time.perf_counter()
    per_iter_ns = (t1 - t0) / iters * 1e9

    res_maps = [
        {n: result[i].reshape(NCORES, *out_avals[i].shape)[c]
         for i, n in enumerate(out_names)}
        for c in range(NCORES)
    ]

    class _R:
        results = res_maps

    return _assemble(_R), per_iter_ns


def _enable_ntff_hook():
    """Synthesize antenv.axon_hooks (absent in this image) and register the
    ctypes NTFF profile hook so run_bass_kernel_spmd(trace=True) works."""
    import sys as _sys
    import types as _types

    if "antenv.axon_hooks" in _sys.modules:
        return
    import antenv  # noqa: F401
    mod = _types.ModuleType("antenv.axon_hooks")
    mod._hook = None

    def set_axon_ntff_profile_hook(h):
        mod._hook = h

    def get_axon_ntff_profile_hook():
        return mod._hook

    mod.set_axon_ntff_profile_hook = set_axon_ntff_profile_hook
    mod.get_axon_ntff_profile_hook = get_axon_ntff_profile_hook
    _sys.modules["antenv.axon_hooks"] = mod
    antenv.axon_hooks = mod
    from trn_agent_boot.trn_boot import _ntff_profile_via_ctypes
    hook = _ntff_profile_via_ctypes("/opt/axon/libaxon_pjrt.so")
    if hook is not None:
        mod.set_axon_ntff_profile_hook(hook)
    # uploads need a fish bucket this container lacks; neuter them
    import concourse.bass_utils as _bu
    _bu.upload_artifacts = lambda tmpdir: f"local:{tmpdir}"


def kernel_traced(x, freqs_cos, freqs_sin, wq, wk, wv, wo, tmpdir=None):
    """Run once with NTFF tracing; returns (output, BassKernelResults)."""
    _enable_ntff_hook()
    in_maps = _host_prep(x, freqs_cos, freqs_sin, wq, wk, wv, wo)
    nc = _get_program()
    res = run_bass_kernel_spmd(
        nc, in_maps, core_ids=list(range(NCORES)), trace=True, tmpdir=tmpdir)
    return _assemble(res), res


# revision 24
# speedup vs baseline: 1.3600x; 1.0497x over previous
"""GQA causal attention (B=1, S=2048, D=4096, H=32, KV=8) on 8 trn2 cores.

Strategy: tensor-parallel over heads for QKV+attention, tensor-parallel
over output columns for the out-projection. Core i owns q-heads 4i..4i+3
and kv-head i. Host pre-transposes weights/x so every matmul contracts
along the partition dim, and pre-permutes wq/wk rows (even|odd
interleave -> [evens;odds]) so RoPE becomes partition-aligned
elementwise math.

Phase 1 (QKV+RoPE) streams x do-tile-major with six concurrent PSUM
accumulation groups so compute starts as soon as the first 128-row
chunk of x/w lands. Attention runs head-locally in a scores^T [t, s]
layout with a two-deep score-matmul pipeline ahead of the exp/mask
chain (exp on Scalar, causal mask multiply on GpSimd, softmax
normalization sums from a ones-vector matmul). Each block's normalized
head outputs are AllGathered (bf16); every core then computes its
512-column shard of the output projection for those tokens using all
32 heads, overlapped so the AllGather of block k hides under the
attention of block k+1 and the out-projection of block k-1. Host
concatenates column shards.

Matmul operands are bf16; accumulation, softmax and RoPE math are fp32.
"""

import sys

import numpy as np

sys.path.insert(0, "/opt/trn_rl_repo")

import ml_dtypes  # noqa: E402

import concourse.bass as bass  # noqa: E402
from concourse import bacc  # noqa: E402
import concourse.mybir as mybir  # noqa: E402
import concourse.tile as tile  # noqa: E402
from concourse.bass_utils import run_bass_kernel_spmd  # noqa: E402

F32 = mybir.dt.float32
BF16 = mybir.dt.bfloat16
NPBF = ml_dtypes.bfloat16

B, S, D = 1, 2048, 4096
H, KV, HD = 32, 8, 128
NCORES = 8
HPC = H // NCORES  # q heads per core = 4
EQ = HPC * HD  # 512 local q features
NE = HPC + 2  # e-tiles per core: 4 q + 1 k + 1 v
SB = 512  # attention s block
NSB = S // SB  # 4
XSB = 512  # phase-1 s sub-block (matmul moving dim)
DO = D // 128  # 32 contraction tiles for qkv projection
EO = (H * HD) // 128  # 32 contraction tiles for out-proj
DSH = D // NCORES  # 512 out-proj columns per core
TT = S // 128  # 16 t-tiles
DC = 4  # phase-1 do-tiles per DMA chunk
RG = [list(range(NCORES))]


def build():
    nc = bacc.Bacc("TRN2", target_bir_lowering=False)
    xt = nc.dram_tensor("xt", [D, S], BF16, kind="ExternalInput")
    wqkvt = nc.dram_tensor("wqkvt", [D, NE * 128], BF16, kind="ExternalInput")
    w3t = nc.dram_tensor("w3t", [H * HD, D], BF16, kind="ExternalInput")
    cc = nc.dram_tensor("cc", [128, S], F32, kind="ExternalInput")
    ss = nc.dram_tensor("ss", [128, S], F32, kind="ExternalInput")
    masks = nc.dram_tensor("masks", [HPC, 128, SB], BF16,
                           kind="ExternalInput")
    ones = nc.dram_tensor("ones", [128, 1], BF16, kind="ExternalInput")
    ident = nc.dram_tensor("ident", [128, 128], BF16, kind="ExternalInput")
    out = nc.dram_tensor("out", [2 * 128, D], BF16, kind="ExternalOutput")

    xt_t = xt[:].rearrange("(do p) s -> p do s", p=128)
    w_t = wqkvt[:].rearrange("(do p) e -> p do e", p=128)
    w3_t = w3t[:].rearrange("(eo p) d -> p eo d", p=128)

    with tile.TileContext(nc) as tc:
        with tc.tile_pool(name="dram", bufs=1, space="DRAM") as dram, \
                tc.tile_pool(name="pqkv", bufs=1) as pqkv:
            # AllToAll buffers: one per sequence half; shard j of a2a_in[g]
            # is [our 512 features, 128 tokens owned by rank j in half g]
            a2a_in = [dram.tile([H * HD, 128], BF16, name=f"a2i{g}")
                      for g in range(2)]
            a2a_out = [dram.tile([H * HD, 128], BF16, name=f"a2o{g}")
                       for g in range(2)]
            qe = [pqkv.tile([128, S], BF16, name=f"qe{et}", tag=f"qe{et}")
                  for et in range(NE)]
            # v in natural [t, hd] tiles (transposed from e-tile layout)
            vn = pqkv.tile([128, TT, HD], BF16)
            idt = pqkv.tile([128, 128], BF16)
            nc.sync.dma_start(idt, ident[:])

            # ---------------- Phase 1: fused QKV projection + RoPE ----------
            with tc.tile_pool(name="p1w", bufs=1) as p1w, \
                    tc.tile_pool(name="p1x", bufs=2) as p1x, \
                    tc.tile_pool(name="p1t", bufs=2) as p1t, \
                    tc.tile_pool(name="p1ps", bufs=1, space="PSUM") as p1ps, \
                    tc.tile_pool(name="p1pt", bufs=2, space="PSUM") as p1pt:
                w = p1w.tile([128, DO, NE * 128], BF16)
                xtiles = []
                # stream first two x blocks and weights in do-chunks so the
                # PE can start on do 0..3 while the rest is in flight; x on
                # the sync queue, w on the scalar queue (parallel arrival)
                for sb in range(2):
                    ssl = slice(sb * XSB, (sb + 1) * XSB)
                    xtile = p1x.tile([128, DO, XSB], BF16, tag="x")
                    # finer chunks up front so the first matmuls start sooner
                    bounds = [0, 2, 4, 8, 12, 16, 20, 24, 28, 32]
                    for lo, hi in zip(bounds, bounds[1:]):
                        dsl = slice(lo, hi)
                        nc.sync.dma_start(xtile[:, dsl, :], xt_t[:, dsl, ssl])
                        if sb == 0:
                            nc.scalar.dma_start(w[:, dsl, :], w_t[:, dsl, :])
                    xtiles.append(xtile)
                for sb in range(NSB):
                    ssl = slice(sb * XSB, (sb + 1) * XSB)
                    if sb < 2:
                        xtile = xtiles[sb]
                    else:
                        xtile = p1x.tile([128, DO, XSB], BF16, tag="x")
                        nc.sync.dma_start(xtile, xt_t[:, :, ssl])
                    cct = p1x.tile([128, XSB], F32, tag="cc")
                    sst = p1x.tile([128, XSB], F32, tag="ss")
                    nc.sync.dma_start(cct, cc[:][:, ssl])
                    nc.sync.dma_start(sst, ss[:][:, ssl])
                    atile = p1t.tile([128, HPC + 1, XSB], F32, tag="at",
                                     bufs=1)
                    btile = p1t.tile([128, HPC + 1, XSB], F32, tag="bt",
                                     bufs=1)
                    bsw = p1t.tile([128, (HPC + 1) * XSB], F32, tag="bsw",
                                   bufs=1)
                    ps = [p1ps.tile([128, XSB], F32, tag=f"ps{et}",
                                    name=f"ps{et}")
                          for et in range(NE)]
                    # do-major: six concurrent PSUM groups; MMs for do-chunk
                    # d need only chunk d of x and w
                    for do in range(DO):
                        for et in range(NE):
                            nc.tensor.matmul(
                                ps[et],
                                w[:, do, et * 128:(et + 1) * 128],
                                xtile[:, do, :],
                                start=(do == 0),
                                stop=(do == DO - 1),
                            )
                    for et in range(NE):
                        if et < HPC + 1:
                            # t*cos and t*sin halves; swap+add finishes RoPE
                            nc.vector.tensor_mul(atile[:, et, :], ps[et], cct)
                            nc.vector.tensor_mul(btile[:, et, :], ps[et], sst)
                        else:
                            nc.any.tensor_copy(qe[et][:, ssl], ps[et])
                    # transpose this s-block's four v t-tiles right away
                    for tt in range(sb * 4, sb * 4 + 4):
                        pst = p1pt.tile([128, 128], BF16, tag="pst")
                        nc.tensor.transpose(
                            pst, qe[HPC + 1][:, tt * 128:(tt + 1) * 128], idt)
                        nc.any.tensor_copy(vn[:, tt, :], pst)
                    bt2 = btile.rearrange("p e s -> p (e s)")
                    nc.sync.dma_start(bsw[:64, :], bt2[64:, :])
                    nc.sync.dma_start(bsw[64:, :], bt2[:64, :])
                    bs3 = bsw.rearrange("p (e s) -> p e s", s=XSB)
                    for et in range(HPC + 1):
                        nc.vector.tensor_add(
                            qe[et][:, ssl], atile[:, et, :], bs3[:, et, :])

            # ------- Phase 2+3: attention blocks + pipelined out-proj -------
            with tc.tile_pool(name="p2c", bufs=1) as p2c, \
                    tc.tile_pool(name="p2e", bufs=4) as p2e, \
                    tc.tile_pool(name="p2t", bufs=2) as p2t, \
                    tc.tile_pool(name="p2a", bufs=4) as p2a, \
                    tc.tile_pool(name="p3a", bufs=2) as p3a, \
                    tc.tile_pool(name="p3o", bufs=2) as p3o, \
                    tc.tile_pool(name="psS", bufs=2, space="PSUM") as psS, \
                    tc.tile_pool(name="pav", bufs=2, space="PSUM") as pavp, \
                    tc.tile_pool(name="psN", bufs=2, space="PSUM") as psN, \
                    tc.tile_pool(name="pop", bufs=2, space="PSUM") as popp:
                # tiny setup tensors first (sync queue) so the vn transposes
                # aren't stuck behind the big w3 stream (scalar queue)
                mt = p2c.tile([128, HPC, SB], BF16)
                nc.sync.dma_start(mt, masks[:].rearrange("m p s -> p m s"))
                on = p2c.tile([128, 1], BF16)
                nc.sync.dma_start(on, ones[:])
                a2a_sb = [p2c.tile([128, EO, 128], BF16, name=f"a2s{g}")
                          for g in range(2)]

                for b in range(NSB):
                    bsl = slice(b * SB, (b + 1) * SB)
                    ntt = 4 * b + 4
                    for h in range(HPC):
                        pa = pavp.tile([128, SB], F32, tag="pav")
                        pn = psN.tile([1, SB], F32, tag="nrm")
                        psc = {}

                        def score(j):
                            # diagonal tile m only contributes s >= 128m
                            off = max(0, (j - 4 * b) * 128)
                            psc[j] = psS.tile([128, SB], F32, tag="sc",
                                              name=f"sc{j}")
                            nc.tensor.matmul(
                                psc[j][:, off:],
                                qe[HPC][:, j * 128:(j + 1) * 128],
                                qe[h][:, b * SB + off:(b + 1) * SB],
                                start=True, stop=True)

                        def consume(j):
                            off = max(0, (j - 4 * b) * 128)
                            ex = p2e.tile([128, SB], BF16, tag="ex")
                            nc.scalar.activation(
                                ex[:, off:], psc.pop(j)[:, off:],
                                mybir.ActivationFunctionType.Exp)
                            if j >= 4 * b:
                                m = j - 4 * b
                                nc.vector.tensor_mul(
                                    ex[:, off:], ex[:, off:],
                                    mt[:, m, off:])
                            nc.tensor.matmul(
                                pa[:, off:], vn[:, j, :], ex[:, off:],
                                start=(j == 0), stop=(j == ntt - 1))
                            nc.tensor.matmul(
                                pn[:, off:], on, ex[:, off:],
                                start=(j == 0), stop=(j == ntt - 1))

                        score(0)
                        if ntt > 1:
                            score(1)
                        for j in range(ntt):
                            if j + 2 < ntt:
                                score(j + 2)
                            consume(j)
                        rc1 = p2t.tile([1, SB], F32, tag="rc1")
                        nc.vector.reciprocal_approx_fast(rc1, pn)
                        rcp = p2t.tile([128, SB], F32, tag="rcp")
                        nc.gpsimd.partition_broadcast(rcp, rc1)
                        avn = p2a.tile([128, SB], BF16, tag="avn")
                        nc.vector.tensor_mul(avn, pa, rcp)
                        # scatter the 4 quarter-columns to their owner
                        # ranks' shards of this half's AllToAll input
                        g = b // 2
                        for k in range(4):
                            shard = 4 * (b % 2) + k
                            nc.sync.dma_start(
                                a2a_in[g][:][shard * EQ + h * 128:
                                             shard * EQ + (h + 1) * 128, :],
                                avn[:, k * 128:(k + 1) * 128])
                    if b % 2 == 1:
                        g = b // 2
                        nc.gpsimd.collective_compute(
                            "AllToAll",
                            mybir.AluOpType.bypass,
                            ins=[a2a_in[g].opt()],
                            outs=[a2a_out[g].opt()],
                            replica_groups=RG,
                        )
                        nc.gpsimd.dma_start(
                            a2a_sb[g],
                            a2a_out[g][:].rearrange("(eo p) s -> p eo s",
                                                    p=128))

                # out-projection: stream full wo in 512-column chunks; each
                # core computes [its 256 tokens, all 4096 d]
                for dbm in range(D // DSH):
                    w3c = p3a.tile([128, EO, DSH], BF16, tag="w3c")
                    nc.sync.dma_start(
                        w3c, w3_t[:, :, dbm * DSH:(dbm + 1) * DSH])
                    for g in range(2):
                        po = popp.tile([128, DSH], F32, tag="pop")
                        for eo in range(EO):
                            nc.tensor.matmul(
                                po,
                                a2a_sb[g][:, eo, :],
                                w3c[:, eo, :],
                                start=(eo == 0), stop=(eo == EO - 1))
                        osb = p3o.tile([128, DSH], BF16, tag="osb")
                        nc.vector.tensor_copy(osb, po)
                        nc.sync.dma_start(
                            out[:][g * 128:(g + 1) * 128,
                                   dbm * DSH:(dbm + 1) * DSH], osb)
    nc.compile()
    return nc


_CACHE = {}


def _get_program():
    if "nc" not in _CACHE:
        _CACHE["nc"] = build()
    return _CACHE["nc"]


def _host_prep(x, freqs_cos, freqs_sin, wq, wk, wv, wo):
    x2 = np.ascontiguousarray(np.asarray(x, np.float32).reshape(S, D))
    xT = np.ascontiguousarray(x2.T).astype(NPBF)
    # even|odd -> [evens;odds] row permutation per head (RoPE partition split)
    perm1 = np.concatenate([np.arange(0, HD, 2), np.arange(1, HD, 2)])
    permq = (np.arange(H)[:, None] * HD + perm1[None, :]).reshape(-1)
    permk = (np.arange(KV)[:, None] * HD + perm1[None, :]).reshape(-1)
    scale = np.float32(1.0 / np.sqrt(HD))
    wq_p = np.asarray(wq, np.float32)[permq] * scale
    wk_p = np.asarray(wk, np.float32)[permk]
    wv32 = np.asarray(wv, np.float32)
    wo32 = np.asarray(wo, np.float32)
    cosT = np.asarray(freqs_cos, np.float32).T
    sinT = np.asarray(freqs_sin, np.float32).T
    ccb = np.ascontiguousarray(np.concatenate([cosT, cosT], 0))
    ssb = np.ascontiguousarray(np.concatenate([sinT, -sinT], 0))
    tp = np.arange(128, dtype=np.int64)[:, None]
    sf = np.arange(SB, dtype=np.int64)[None, :]
    masks = np.stack(
        [(sf >= tp + 128 * m).astype(NPBF) for m in range(HPC)], 0)
    ones = np.ones((128, 1), NPBF)
    ident = np.eye(128, dtype=NPBF)

    w3t = np.ascontiguousarray(wo32.T).astype(NPBF)  # [4096 e, 4096 d]
    in_maps = []
    for i in range(NCORES):
        wqkv = np.concatenate(
            [wq_p[i * EQ:(i + 1) * EQ],
             wk_p[i * HD:(i + 1) * HD],
             wv32[i * HD:(i + 1) * HD]], 0)
        wqkvt = np.ascontiguousarray(wqkv.T).astype(NPBF)
        in_maps.append(dict(xt=xT, wqkvt=wqkvt, w3t=w3t, cc=ccb, ss=ssb,
                            masks=masks, ones=ones, ident=ident))
    return in_maps


def _run(in_maps, trace=False):
    nc = _get_program()
    return run_bass_kernel_spmd(
        nc, in_maps, core_ids=list(range(NCORES)), trace=trace)


def _assemble(res):
    full = np.empty((S, D), np.float32)
    for r in range(NCORES):
        shard = np.asarray(res.results[r]["out"]).astype(np.float32)
        full[r * 128:(r + 1) * 128, :] = shard[:128]
        full[S // 2 + r * 128:S // 2 + (r + 1) * 128, :] = shard[128:]
    return full.reshape(B, S, D)


def kernel(x, freqs_cos, freqs_sin, wq, wk, wv, wo):
    in_maps = _host_prep(x, freqs_cos, freqs_sin, wq, wk, wv, wo)
    res = _run(in_maps, trace=False)
    return _assemble(res)


def _build_sharded():
    """Mirror of bass2jax.run_bass_via_pjrt's multi-core path, split so the
    jitted callable and device-resident inputs can be reused for timing."""
    import jax
    from jax.experimental.shard_map import shard_map
    from jax.sharding import Mesh, PartitionSpec

    import concourse.mybir as mb
    from concourse import bass2jax

    nc = _get_program()
    bass2jax.install_neuronx_cc_hook()
    part_name = (nc.partition_id_tensor.name
                 if nc.partition_id_tensor else None)
    in_names, out_names, out_avals, zero_outs = [], [], [], []
    for alloc in nc.m.functions[0].allocations:
        if not isinstance(alloc, mb.MemoryLocationSet):
            continue
        name = alloc.memorylocations[0].name
        if alloc.kind == "ExternalInput":
            if name != part_name:
                in_names.append(name)
        elif alloc.kind == "ExternalOutput":
            out_names.append(name)
            shape = tuple(alloc.tensor_shape)
            dtype = mb.dt.np(alloc.dtype)
            out_avals.append(jax.core.ShapedArray(shape, dtype))
            zero_outs.append(np.zeros(shape, dtype))
    n_params = len(in_names)
    all_names = in_names + out_names
    if part_name is not None:
        all_names = all_names + [part_name]

    def _body(*args):
        operands = list(args)
        if part_name is not None:
            operands.append(bass2jax.partition_id_tensor())
        outs = bass2jax._bass_exec_p.bind(
            *operands,
            out_avals=tuple(out_avals),
            in_names=tuple(all_names),
            out_names=tuple(out_names),
            lowering_input_output_aliases=(),
            sim_require_finite=True,
            sim_require_nnan=True,
            nc=nc,
        )
        return tuple(outs)

    devices = jax.devices()[:NCORES]
    mesh = Mesh(np.asarray(devices), ("core",))
    n_outs = len(out_names)
    sharded = jax.jit(
        shard_map(
            _body, mesh=mesh,
            in_specs=(PartitionSpec("core"),) * (n_params + n_outs),
            out_specs=(PartitionSpec("core"),) * n_outs,
            check_rep=False,
        ),
        donate_argnums=tuple(range(n_params, n_params + n_outs)),
        keep_unused=True,
    )
    return sharded, in_names, out_names, out_avals, zero_outs, mesh


def kernel_profiled(x, freqs_cos, freqs_sin, wq, wk, wv, wo, iters=12):
    """Returns (output, per-execution wall ns). Times repeated on-device
    executions with inputs pre-placed on the devices."""
    import time

    import jax
    from jax.sharding import NamedSharding, PartitionSpec

    in_maps = _host_prep(x, freqs_cos, freqs_sin, wq, wk, wv, wo)
    sharded, in_names, out_names, out_avals, zero_outs, mesh = _build_sharded()
    spec = NamedSharding(mesh, PartitionSpec("core"))
    concat_in = [
        jax.device_put(
            np.concatenate([in_maps[c][n] for c in range(NCORES)], axis=0),
            spec)
        for n in in_names
    ]

    def zeros():
        return [
            jax.device_put(
                np.zeros((NCORES * z.shape[0], *z.shape[1:]), z.dtype), spec)
            for z in zero_outs
        ]

    out_arrs = sharded(*concat_in, *zeros())  # warmup & result
    jax.block_until_ready(out_arrs)
    result = [np.asarray(a) for a in out_arrs]

    zsets = [zeros() for _ in range(iters)]
    jax.block_until_ready(zsets)
    t0 = time.perf_counter()
    last = None
    for zs in zsets:
        last = sharded(*concat_in, *zs)
    jax.block_until_ready(last)
    t1 = time.perf_counter()
    per_iter_ns = (t1 - t0) / iters * 1e9

    res_maps = [
        {n: result[i].reshape(NCORES, *out_avals[i].shape)[c]
         for i, n in enumerate(out_names)}
        for c in range(NCORES)
    ]

    class _R:
        results = res_maps

    return _assemble(_R), per_iter_ns


def _enable_ntff_hook():
    """Synthesize antenv.axon_hooks (absent in this image) and register the
    ctypes NTFF profile hook so run_bass_kernel_spmd(trace=True) works."""
    import sys as _sys
    import types as _types

    if "antenv.axon_hooks" in _sys.modules:
        return
    import antenv  # noqa: F401
    mod = _types.ModuleType("antenv.axon_hooks")
    mod._hook = None

    def set_axon_ntff_profile_hook(h):
        mod._hook = h

    def get_axon_ntff_profile_hook():
        return mod._hook

    mod.set_axon_ntff_profile_hook = set_axon_ntff_profile_hook
    mod.get_axon_ntff_profile_hook = get_axon_ntff_profile_hook
    _sys.modules["antenv.axon_hooks"] = mod
    antenv.axon_hooks = mod
    from trn_agent_boot.trn_boot import _ntff_profile_via_ctypes
    hook = _ntff_profile_via_ctypes("/opt/axon/libaxon_pjrt.so")
    if hook is not None:
        mod.set_axon_ntff_profile_hook(hook)
    # uploads need a fish bucket this container lacks; neuter them
    import concourse.bass_utils as _bu
    _bu.upload_artifacts = lambda tmpdir: f"local:{tmpdir}"


def kernel_traced(x, freqs_cos, freqs_sin, wq, wk, wv, wo, tmpdir=None):
    """Run once with NTFF tracing; returns (output, BassKernelResults)."""
    _enable_ntff_hook()
    in_maps = _host_prep(x, freqs_cos, freqs_sin, wq, wk, wv, wo)
    nc = _get_program()
    res = run_bass_kernel_spmd(
        nc, in_maps, core_ids=list(range(NCORES)), trace=True, tmpdir=tmpdir)
    return _assemble(res), res
